# revision 31
# baseline (speedup 1.0000x reference)
"""BipartiteGCN message-passing kernel for 8 TRN2 NeuronCores.

Math:  out = D_c^{-1/2} A^T D_r^{-1/2} (x @ W) + b
where A[s, d] = multiplicity of edge (gene s, drug d), s, d in [0, 4000).

Strategy (gene-window sharding, single f16 ReduceScatter):
  - Core c owns gene window [512c, 512c+512).  It holds ALL edges whose src
    falls in its window, so row_deg is local (no collective needed for f).
  - xw_c = x_c @ W computed locally with bf16 matmuls (1 cyc/row, inputs
    host-cast to bf16), then scaled by f = rsqrt(max(row_deg,1)) per gene.
  - A_c [512 genes x 4096 drugs] built in SBUF from one-hot outer products
    on the PE.  Edges are bucketed by (gene subwindow gs in 4, drug window
    dw in 8); within a bucket they are sorted by dst and cut into <=128-edge
    chunks at dst-value boundaries.  Cut points are computed from the
    max-over-cores running counts, so all 8 cores share one SPMD module;
    chunk dst-spans tile [0,512) disjointly, so every A-build matmul is its
    own start&stop accumulation region (no psum pre-zeroing).  One-hot
    compares (DVE 4x mode, 0.26 ns/elem) are split between DVE and Pool.
  - P_c = A_c^T @ (f*xw_c) partials [4096 drugs x 512] via fp8e4 DoubleRow
    matmuls (2 gene layers per matmul at 0.5 cyc/row) with an fp8 residual
    correction pass (P += A^T @ fp8(xwf - fp8(xwf))) to recover precision;
    col_deg partial rows are packed into one staged [8*513, 512] f16 tensor;
    a single f16 ReduceScatter sums partials and hands core c its 513-row
    stripe (512 P rows + 1 col_deg row).
  - Post: g = rsqrt(max(col_deg,1)), out = g*P + bias; host concatenates.
    (Zero-degree genes/drugs have all-zero A rows/cols, so no rsqrt masks.)
"""

import sys

if "/opt/trn_rl_repo" not in sys.path:
    sys.path.insert(0, "/opt/trn_rl_repo")

import numpy as np

import concourse.bass as bass  # noqa: F401
import concourse.mybir as mybir
from concourse import bacc, tile

CORES = 8
ND = 4000               # number of drugs (dst ids; src gene ids share range)
NDP = 4096              # padded drug dim
GW = 512                # genes per core
NGS = 4                 # gene subwindows of 128
NDW = 8                 # drug windows of 512
IC = 1024
OC = 512
SROW = NDW * 513        # staged rows: per window 512 P rows + 1 coldeg row

F32 = mybir.dt.float32
F16 = mybir.dt.float16
BF16 = mybir.dt.bfloat16
F8 = mybir.dt.float8e4


def build_structure(edge_index):
    """Compile-time chunk structure shared by all cores (SPMD).

    Returns chunks: list of (gs, dw, lo, hi) in bucket order (gs outer,
    dw inner), where every core has <=128 edges with src in its (c, gs)
    subwindow and dst-local in [lo, hi).
    """
    s_all = np.asarray(edge_index[0], dtype=np.int64)
    d_all = np.asarray(edge_index[1], dtype=np.int64)
    assert s_all.min() >= 0 and s_all.max() < ND, "src ids out of range"
    assert d_all.min() >= 0 and d_all.max() < ND, "dst ids out of range"

    core = s_all >> 9                  # gene window of 512
    gs = (s_all >> 7) & 3              # subwindow of 128
    dw = d_all >> 9                    # drug window of 512
    dl = d_all & 511                   # dst-local
    # counts[core, gs, dw, dl]
    key = ((core * NGS + gs) * NDW + dw) * 512 + dl
    counts = np.bincount(key, minlength=CORES * NGS * NDW * 512).reshape(
        CORES, NGS, NDW, 512
    )

    chunks = []
    for g in range(NGS):
        for w in range(NDW):
            cnt = counts[:, g, w, :]            # [CORES, 512]
            assert cnt.max() <= 128, "single dst value multiplicity > 128"
            lo = 0
            acc = np.zeros(CORES, dtype=np.int64)
            for v in range(512):
                cv = cnt[:, v]
                if (acc + cv).max() > 128:
                    chunks.append((g, w, lo, v))
                    lo = v
                    acc = cv.copy()
                else:
                    acc += cv
            chunks.append((g, w, lo, 512))
    return chunks


def build_nc(chunks):
    nch = len(chunks)
    # first/last global chunk index per gs (for rowdeg psum start/stop)
    gs_first = {}
    gs_last = {}
    for i, (g, w, lo, hi) in enumerate(chunks):
        gs_first.setdefault(g, i)
        gs_last[g] = i

    nc = bacc.Bacc(
        None,
        target_bir_lowering=False,
        debug=False,
        num_devices=CORES,
    )

    xT = nc.dram_tensor("xT", [IC, GW], BF16, kind="ExternalInput")
    w_in = nc.dram_tensor("w", [IC, OC], BF16, kind="ExternalInput")
    brep = nc.dram_tensor("brep", [128, OC], F32, kind="ExternalInput")
    i128 = nc.dram_tensor("i128", [128, 128], F16, kind="ExternalInput")
    i512 = nc.dram_tensor("i512", [128, 512], F16, kind="ExternalInput")
    ident = nc.dram_tensor("ident", [128, 128], F32, kind="ExternalInput")
    sloc = nc.dram_tensor("sloc", [128, nch], F32, kind="ExternalInput")
    dloc = nc.dram_tensor("dloc", [128, nch], F32, kind="ExternalInput")
    out = nc.dram_tensor("out", [GW, OC], F32, kind="ExternalOutput")

    staged = nc.dram_tensor("staged", [SROW, OC], F16)
    rsout = nc.dram_tensor("rsout", [513, OC], F16)

    with tile.TileContext(nc) as tc:
        with (
            tc.tile_pool(name="const", bufs=1) as cpool,
            tc.tile_pool(name="work", bufs=2) as wpool,
            tc.tile_pool(name="apool", bufs=1) as apool,
            tc.tile_pool(name="psum", bufs=2, space="PSUM") as ppool,
            tc.tile_pool(name="psheld", bufs=1, space="PSUM") as hpool,
        ):
            # ---- constants (sloc/dloc first, spread across SP/ACT) ----
            sloc_sb = cpool.tile([128, nch], F32)
            nc.sync.dma_start(sloc_sb[:], sloc[:])
            dloc_sb = cpool.tile([128, nch], F32)
            nc.scalar.dma_start(dloc_sb[:], dloc[:])
            i128_sb = cpool.tile([128, 128], F16)
            nc.sync.dma_start(i128_sb[:], i128[:])
            i512_sb = cpool.tile([128, 512], F16)
            nc.sync.dma_start(i512_sb[:], i512[:])
            ident_sb = cpool.tile([128, 128], F32)
            nc.scalar.dma_start(ident_sb[:], ident[:])
            bias_sb = cpool.tile([128, OC], F32)
            nc.scalar.dma_start(bias_sb[:], brep[:])
            # preload the ACT sqrt function table off the critical path
            junk = cpool.tile([128, 1], F32)
            nc.vector.memset(junk[:], 4.0)
            nc.scalar.sqrt(junk[:], junk[:])
            ones16 = cpool.tile([128, 1], F16)
            nc.vector.memset(ones16[:], 1.0)
            ones_f8 = cpool.tile([128, 1], F8)
            nc.vector.memset(ones_f8[:], 1.0)

            # ---- phase B: xw = x_c @ W (bf16) ----
            xt_t = []
            w_t = []
            for kt in range(8):
                xt = wpool.tile([128, GW], BF16, tag="xT", bufs=8,
                                name=f"xt{kt}")
                wt = wpool.tile([128, OC], BF16, tag="w", bufs=8,
                                name=f"w{kt}")
                nc.sync.dma_start(xt[:], xT[kt * 128:(kt + 1) * 128, :])
                nc.sync.dma_start(wt[:], w_in[kt * 128:(kt + 1) * 128, :])
                xt_t.append(xt)
                w_t.append(wt)
            xw_sb = []
            for gs in range(NGS):
                pg = ppool.tile([128, OC], F32, tag="pg", bufs=1,
                                name=f"pg{gs}")
                for kt in range(8):
                    nc.tensor.matmul(
                        pg[:],
                        xt_t[kt][:, gs * 128:(gs + 1) * 128],
                        w_t[kt][:],
                        start=(kt == 0),
                        stop=(kt == 7),
                    )
                t = cpool.tile([128, OC], F32, name=f"xw{gs}")
                nc.scalar.copy(t[:], pg[:])
                xw_sb.append(t)

            # ---- phase D: A-build + rowdeg ----
            a_sb = [apool.tile([128, 2 * NDP], F8, name=f"A{pr}") for pr in range(2)]
            acc_ps = hpool.tile([128, 36], F32, name="accps")
            f_sb = cpool.tile([128, NGS], F32)
            xwf_sb = [cpool.tile([128, 2 * OC], F8, name=f"xwf{pr}")
                      for pr in range(2)]
            xwr_sb = [cpool.tile([128, 2 * OC], F8, name=f"xwr{pr}")
                      for pr in range(2)]
            ci = 0
            bi = 0
            for g in range(NGS):
                for w in range(NDW):
                    pa = ppool.tile([128, 512], F32, tag="pa", bufs=3,
                                    name=f"pa{bi}")
                    bspans = [c for c in chunks if c[0] == g and c[1] == w]
                    for (gg, ww, lo, hi) in bspans:
                        c = ci
                        ci += 1
                        loh = wpool.tile([128, 128], F16, tag="loh", bufs=32,
                                         name=f"loh{c}")
                        roh = wpool.tile([128, 512], F16, tag="roh", bufs=32,
                                         name=f"roh{c}")
                        eng_a = nc.vector if (c & 1) == 0 else nc.gpsimd
                        eng_b = nc.gpsimd if (c & 1) == 0 else nc.vector
                        eng_a.tensor_scalar(
                            out=loh[:], in0=i128_sb[:],
                            scalar1=sloc_sb[:, c:c + 1], scalar2=None,
                            op0=mybir.AluOpType.is_equal,
                        )
                        eng_b.tensor_scalar(
                            out=roh[:, 0:hi - lo], in0=i512_sb[:, lo:hi],
                            scalar1=dloc_sb[:, c:c + 1], scalar2=None,
                            op0=mybir.AluOpType.is_equal,
                        )
                        nc.tensor.matmul(
                            pa[:, lo:hi], loh[:], roh[:, 0:hi - lo],
                            start=True, stop=True,
                        )
                        nc.tensor.matmul(
                            acc_ps[:, 32 + g:33 + g], loh[:], ones16[:],
                            start=(c == gs_first[g]), stop=(c == gs_last[g]),
                        )
                    nc.scalar.copy(
                        a_sb[g // 2][:, (g % 2) * NDP + w * 512:
                                     (g % 2) * NDP + (w + 1) * 512],
                        pa[:])
                    bi += 1
                # rowdeg for this gs group just closed: f[g] and xwf[g]
                # now, hidden under the remaining groups' build (zero-deg
                # genes have all-zero A rows, so f needs no mask)
                nc.vector.tensor_scalar(
                    out=f_sb[:, g:g + 1], in0=acc_ps[:, 32 + g:33 + g],
                    scalar1=1.0, scalar2=None,
                    op0=mybir.AluOpType.max,
                )
                nc.scalar.sqrt(f_sb[:, g:g + 1], f_sb[:, g:g + 1])
                nc.vector.reciprocal(f_sb[:, g:g + 1], f_sb[:, g:g + 1])
                nc.vector.tensor_scalar(
                    out=xw_sb[g][:], in0=xw_sb[g][:],
                    scalar1=f_sb[:, g:g + 1], scalar2=None,
                    op0=mybir.AluOpType.mult,
                )
                xf8 = xwf_sb[g // 2][:, (g % 2) * OC:(g % 2 + 1) * OC]
                nc.vector.tensor_copy(xf8, xw_sb[g][:])
                nc.vector.tensor_tensor(
                    out=xwr_sb[g // 2][:, (g % 2) * OC:(g % 2 + 1) * OC],
                    in0=xw_sb[g][:], in1=xf8,
                    op=mybir.AluOpType.subtract,
                )

            # coldeg matmuls first (tiny; A ready), transpose, stage early
            for w in range(NDW):
                for q in range(4):
                    col = w * 4 + q
                    off = w * 512 + q * 128
                    for gs in range(NGS):
                        nc.tensor.matmul(
                            acc_ps[:, col:col + 1],
                            a_sb[gs // 2][:, (gs % 2) * NDP + off:
                                          (gs % 2) * NDP + off + 128],
                            ones_f8[:],
                            start=(gs == 0),
                            stop=(gs == 3),
                        )
            cd_sb = cpool.tile([128, 32], F32)
            nc.vector.tensor_copy(cd_sb[:], acc_ps[:, 0:32])
            cdT_ps = ppool.tile([32, 128], F32, tag="cdT", bufs=1)
            nc.tensor.transpose(cdT_ps[:], cd_sb[:], ident_sb[:])
            cdT16 = cpool.tile([32, 128], F16)
            nc.vector.tensor_copy(cdT16[:], cdT_ps[:])
            nc.gpsimd.dma_start(
                staged[512::513, :].rearrange("a (b f) -> a b f", b=4),
                cdT16[:],
            )

            # ---- phase F: P = A^T @ xwf ----
            for w in range(NDW):
                p16 = wpool.tile([128, 4 * OC], F16, tag="p16", bufs=3,
                                 name=f"p16_{w}")
                for q in range(4):
                    pp = ppool.tile([128, OC], F32, tag="pp", bufs=2,
                                    name=f"pp{w}_{q}")
                    off = w * 512 + q * 128
                    for pr in range(2):
                        lhsT = a_sb[pr][:].rearrange(
                            "p (two d) -> p two d", two=2
                        )[:, :, off:off + 128]
                        for si, srcs in enumerate((xwf_sb, xwr_sb)):
                            rhs = srcs[pr][:].rearrange(
                                "p (two n) -> p two n", two=2
                            )
                            nc.tensor.matmul(
                                pp[:], lhsT, rhs,
                                start=(pr == 0 and si == 0),
                                stop=(pr == 1 and si == 1),
                                perf_mode=mybir.MatmulPerfMode.DoubleRow,
                            )
                    nc.scalar.copy(p16[:, q * OC:(q + 1) * OC], pp[:])
                # one batched DMA per window: rows 513w + q*128 + p
                dstv = staged[513 * w:513 * w + 512, :].rearrange(
                    "(q p) j -> p q j", q=4
                )
                nc.sync.dma_start(dstv, p16[:])

            # ---- ReduceScatter (sums partials, core c gets its stripe) ----
            nc.gpsimd.collective_compute(
                "ReduceScatter",
                mybir.AluOpType.add,
                replica_groups=[list(range(CORES))],
                ins=[staged[:].opt()],
                outs=[rsout[:].opt()],
            )

            # ---- post: g scale + bias (zero-coldeg drugs have P=0) ----
            cdg16 = cpool.tile([128, 4], F16)
            nc.gpsimd.dma_start(
                cdg16[:],
                rsout[512:513, :].rearrange("r (q p) -> (r p) q", q=4),
            )
            g_sb = cpool.tile([128, 4], F32)
            nc.vector.tensor_scalar(
                out=g_sb[:], in0=cdg16[:], scalar1=1.0, scalar2=None,
                op0=mybir.AluOpType.max,
            )
            nc.scalar.sqrt(g_sb[:], g_sb[:])
            nc.vector.reciprocal(g_sb[:], g_sb[:])
            for q in range(4):
                pq = wpool.tile([128, OC], F16, tag="pq", bufs=4,
                                name=f"pq{q}")
                eng = nc.sync if q % 2 == 0 else nc.gpsimd
                eng.dma_start(pq[:], rsout[q * 128:(q + 1) * 128, :])
                og = wpool.tile([128, OC], F32, tag="og", bufs=4,
                                name=f"og{q}")
                nc.vector.scalar_tensor_tensor(
                    out=og[:], in0=pq[:], scalar=g_sb[:, q:q + 1],
                    in1=bias_sb[:],
                    op0=mybir.AluOpType.mult, op1=mybir.AluOpType.add,
                )
                oeng = nc.gpsimd if q % 2 == 0 else nc.sync
                oeng.dma_start(out[q * 128:(q + 1) * 128, :], og[:])

    nc.finalize()
    return nc


def make_in_maps(x, weight, bias, edge_index, chunks):
    """Host-side sharding/layout only: no arithmetic on tensor values."""
    x = np.asarray(x, dtype=np.float32)
    weight = np.ascontiguousarray(np.asarray(weight, dtype=np.float32))
    bias = np.asarray(bias, dtype=np.float32)
    ei = np.asarray(edge_index)
    s_all = ei[0].astype(np.int64)
    d_all = ei[1].astype(np.int64)

    nch = len(chunks)
    brep = np.ascontiguousarray(
        np.tile(bias[None, :], (128, 1)).astype(np.float32)
    )
    i128 = np.ascontiguousarray(
        np.tile(np.arange(128, dtype=np.float16)[None, :], (128, 1))
    )
    i512 = np.ascontiguousarray(
        np.tile(np.arange(512, dtype=np.float16)[None, :], (128, 1))
    )
    ident = np.eye(128, dtype=np.float32)

    in_maps = []
    for c in range(CORES):
        m = (s_all >= GW * c) & (s_all < GW * (c + 1))
        s = s_all[m] - GW * c          # [0, 512)
        d = d_all[m]
        gs = s >> 7
        dw = d >> 9
        dl = d & 511
        sl_arr = np.full((128, nch), -1.0, dtype=np.float32)
        dl_arr = np.full((128, nch), -1.0, dtype=np.float32)
        # order edges to match chunk structure
        for t, (g, w, lo, hi) in enumerate(chunks):
            sel = (gs == g) & (dw == w) & (dl >= lo) & (dl < hi)
            n = int(sel.sum())
            assert n <= 128, f"chunk overflow: {n}"
            sl_arr[:n, t] = (s[sel] - 128 * g).astype(np.float32)
            dl_arr[:n, t] = dl[sel].astype(np.float32)

        import ml_dtypes

        xsT = np.ascontiguousarray(
            x[GW * c:GW * (c + 1), :].T.astype(ml_dtypes.bfloat16)
        )

        in_maps.append(
            {
                "xT": xsT,
                "w": np.ascontiguousarray(weight.astype(ml_dtypes.bfloat16)),
                "brep": brep,
                "i128": i128,
                "i512": i512,
                "ident": ident,
                "sloc": np.ascontiguousarray(sl_arr),
                "dloc": np.ascontiguousarray(dl_arr),
            }
        )
    return in_maps


_NC = None
_CHUNKS = None
_KEY = None


def _get_nc(edge_index):
    global _NC, _CHUNKS, _KEY
    key = hash(np.asarray(edge_index).tobytes())
    if _NC is None or key != _KEY:
        _CHUNKS = build_structure(edge_index)
        _NC = build_nc(_CHUNKS)
        _KEY = key
    return _NC, _CHUNKS


def kernel(x, weight, bias, edge_index, **run_kwargs):
    from concourse.bass_utils import run_bass_kernel_spmd

    nc, chunks = _get_nc(edge_index)
    in_maps = make_in_maps(x, weight, bias, edge_index, chunks)
    res = run_bass_kernel_spmd(nc, in_maps, core_ids=list(range(CORES)),
                               **run_kwargs)
    outs = res.results if hasattr(res, "results") else res
    full = np.empty((NDP, OC), dtype=np.float32)
    for c in range(CORES):
        full[GW * c:GW * (c + 1)] = outs[c]["out"]
    full = full[:ND]
    if run_kwargs:
        return full, res
    return full


# revision 32
# speedup vs baseline: 1.0180x; 1.0180x over previous
"""BipartiteGCN message-passing kernel for 8 TRN2 NeuronCores.

Math:  out = D_c^{-1/2} A^T D_r^{-1/2} (x @ W) + b
where A[s, d] = multiplicity of edge (gene s, drug d), s, d in [0, 4000).

Strategy (gene-window sharding, single f16 ReduceScatter):
  - Core c owns gene window [512c, 512c+512).  It holds ALL edges whose src
    falls in its window, so row_deg is local (no collective needed for f).
  - xw_c = x_c @ W computed locally with bf16 matmuls (1 cyc/row, inputs
    host-cast to bf16), then scaled by f = rsqrt(max(row_deg,1)) per gene.
  - A_c [512 genes x 4096 drugs] built in SBUF from one-hot outer products
    on the PE.  Edges are bucketed by (gene subwindow gs in 4, drug window
    dw in 8); within a bucket they are sorted by dst and cut into <=128-edge
    chunks at dst-value boundaries.  Cut points are computed from the
    max-over-cores running counts, so all 8 cores share one SPMD module;
    chunk dst-spans tile [0,512) disjointly, so every A-build matmul is its
    own start&stop accumulation region (no psum pre-zeroing).  One-hot
    compares (DVE 4x mode, 0.26 ns/elem) are split between DVE and Pool.
  - P_c = A_c^T @ (f*xw_c) partials [4096 drugs x 512] via fp8e4 DoubleRow
    matmuls (2 gene layers per matmul at 0.5 cyc/row) with an fp8 residual
    correction pass (P += A^T @ fp8(xwf - fp8(xwf))) to recover precision;
    col_deg partial rows are packed into one staged [8*513, 512] f16 tensor;
    a single f16 ReduceScatter sums partials and hands core c its 513-row
    stripe (512 P rows + 1 col_deg row).
  - Post: g = rsqrt(max(col_deg,1)), out = g*P + bias; host concatenates.
    (Zero-degree genes/drugs have all-zero A rows/cols, so no rsqrt masks.)
"""

import sys

if "/opt/trn_rl_repo" not in sys.path:
    sys.path.insert(0, "/opt/trn_rl_repo")

import numpy as np

import concourse.bass as bass  # noqa: F401
import concourse.mybir as mybir
from concourse import bacc, tile

CORES = 8
ND = 4000               # number of drugs (dst ids; src gene ids share range)
NDP = 4096              # padded drug dim
GW = 512                # genes per core
NGS = 4                 # gene subwindows of 128
NDW = 8                 # drug windows of 512
IC = 1024
OC = 512
SROW = NDW * 513        # staged rows: per window 512 P rows + 1 coldeg row

F32 = mybir.dt.float32
F16 = mybir.dt.float16
BF16 = mybir.dt.bfloat16
F8 = mybir.dt.float8e4


def build_structure(edge_index):
    """Compile-time chunk structure shared by all cores (SPMD).

    Returns chunks: list of (gs, dw, lo, hi) in bucket order (gs outer,
    dw inner), where every core has <=128 edges with src in its (c, gs)
    subwindow and dst-local in [lo, hi).
    """
    s_all = np.asarray(edge_index[0], dtype=np.int64)
    d_all = np.asarray(edge_index[1], dtype=np.int64)
    assert s_all.min() >= 0 and s_all.max() < ND, "src ids out of range"
    assert d_all.min() >= 0 and d_all.max() < ND, "dst ids out of range"

    core = s_all >> 9                  # gene window of 512
    gs = (s_all >> 7) & 3              # subwindow of 128
    dw = d_all >> 9                    # drug window of 512
    dl = d_all & 511                   # dst-local
    # counts[core, gs, dw, dl]
    key = ((core * NGS + gs) * NDW + dw) * 512 + dl
    counts = np.bincount(key, minlength=CORES * NGS * NDW * 512).reshape(
        CORES, NGS, NDW, 512
    )

    chunks = []
    for g in range(NGS):
        for w in range(NDW):
            cnt = counts[:, g, w, :]            # [CORES, 512]
            assert cnt.max() <= 128, "single dst value multiplicity > 128"
            lo = 0
            acc = np.zeros(CORES, dtype=np.int64)
            for v in range(512):
                cv = cnt[:, v]
                if (acc + cv).max() > 128:
                    chunks.append((g, w, lo, v))
                    lo = v
                    acc = cv.copy()
                else:
                    acc += cv
            chunks.append((g, w, lo, 512))
    return chunks


def build_nc(chunks):
    nch = len(chunks)
    # first/last global chunk index per gs (for rowdeg psum start/stop)
    gs_first = {}
    gs_last = {}
    for i, (g, w, lo, hi) in enumerate(chunks):
        gs_first.setdefault(g, i)
        gs_last[g] = i

    nc = bacc.Bacc(
        None,
        target_bir_lowering=False,
        debug=False,
        num_devices=CORES,
    )

    xT = nc.dram_tensor("xT", [IC, GW], BF16, kind="ExternalInput")
    w_in = nc.dram_tensor("w", [IC, OC], BF16, kind="ExternalInput")
    brep = nc.dram_tensor("brep", [128, OC], F32, kind="ExternalInput")
    i128 = nc.dram_tensor("i128", [128, 128], F16, kind="ExternalInput")
    i512 = nc.dram_tensor("i512", [128, 512], F16, kind="ExternalInput")
    ident = nc.dram_tensor("ident", [128, 128], F32, kind="ExternalInput")
    sloc = nc.dram_tensor("sloc", [128, nch], F32, kind="ExternalInput")
    dloc = nc.dram_tensor("dloc", [128, nch], F32, kind="ExternalInput")
    out = nc.dram_tensor("out", [GW, OC], F32, kind="ExternalOutput")

    staged = nc.dram_tensor("staged", [SROW, OC], F16)
    rsout = nc.dram_tensor("rsout", [513, OC], F16)

    with tile.TileContext(nc) as tc:
        with (
            tc.tile_pool(name="const", bufs=1) as cpool,
            tc.tile_pool(name="work", bufs=2) as wpool,
            tc.tile_pool(name="apool", bufs=1) as apool,
            tc.tile_pool(name="psum", bufs=2, space="PSUM") as ppool,
            tc.tile_pool(name="psheld", bufs=1, space="PSUM") as hpool,
        ):
            # ---- constants (sloc/dloc first, spread across SP/ACT) ----
            sloc_sb = cpool.tile([128, nch], F32)
            nc.sync.dma_start(sloc_sb[:], sloc[:])
            dloc_sb = cpool.tile([128, nch], F32)
            nc.scalar.dma_start(dloc_sb[:], dloc[:])
            i128_sb = cpool.tile([128, 128], F16)
            nc.sync.dma_start(i128_sb[:], i128[:])
            i512_sb = cpool.tile([128, 512], F16)
            nc.sync.dma_start(i512_sb[:], i512[:])
            ident_sb = cpool.tile([128, 128], F32)
            nc.scalar.dma_start(ident_sb[:], ident[:])
            bias_sb = cpool.tile([128, OC], F32)
            nc.scalar.dma_start(bias_sb[:], brep[:])
            # preload the ACT sqrt function table off the critical path
            junk = cpool.tile([128, 1], F32)
            nc.vector.memset(junk[:], 4.0)
            nc.scalar.sqrt(junk[:], junk[:])
            ones16 = cpool.tile([128, 1], F16)
            nc.vector.memset(ones16[:], 1.0)
            ones_f8 = cpool.tile([128, 1], F8)
            nc.vector.memset(ones_f8[:], 1.0)

            # ---- phase B: xw = x_c @ W (bf16) ----
            xt_t = []
            w_t = []
            for kt in range(8):
                xt = wpool.tile([128, GW], BF16, tag="xT", bufs=8,
                                name=f"xt{kt}")
                wt = wpool.tile([128, OC], BF16, tag="w", bufs=8,
                                name=f"w{kt}")
                nc.sync.dma_start(xt[:], xT[kt * 128:(kt + 1) * 128, :])
                nc.sync.dma_start(wt[:], w_in[kt * 128:(kt + 1) * 128, :])
                xt_t.append(xt)
                w_t.append(wt)
            xw_sb = []
            for gs in range(NGS):
                pg = ppool.tile([128, OC], F32, tag="pg", bufs=1,
                                name=f"pg{gs}")
                for kt in range(8):
                    nc.tensor.matmul(
                        pg[:],
                        xt_t[kt][:, gs * 128:(gs + 1) * 128],
                        w_t[kt][:],
                        start=(kt == 0),
                        stop=(kt == 7),
                    )
                t = cpool.tile([128, OC], F32, name=f"xw{gs}")
                nc.scalar.copy(t[:], pg[:])
                xw_sb.append(t)

            # ---- phase D: A-build + rowdeg ----
            a_sb = [apool.tile([128, 2 * NDP], F8, name=f"A{pr}") for pr in range(2)]
            acc_ps = hpool.tile([128, 36], F32, name="accps")
            f_sb = cpool.tile([128, NGS], F32)
            xwf_sb = [cpool.tile([128, 2 * OC], F8, name=f"xwf{pr}")
                      for pr in range(2)]
            xwr_sb = [cpool.tile([128, 2 * OC], F8, name=f"xwr{pr}")
                      for pr in range(2)]
            ci = 0
            bi = 0
            for g in range(NGS):
                for w in range(NDW):
                    pa = ppool.tile([128, 512], F32, tag="pa", bufs=3,
                                    name=f"pa{bi}")
                    bspans = [c for c in chunks if c[0] == g and c[1] == w]
                    for (gg, ww, lo, hi) in bspans:
                        c = ci
                        ci += 1
                        loh = wpool.tile([128, 128], F16, tag="loh", bufs=32,
                                         name=f"loh{c}")
                        roh = wpool.tile([128, 512], F16, tag="roh", bufs=32,
                                         name=f"roh{c}")
                        eng_a = nc.vector if (c & 1) == 0 else nc.gpsimd
                        eng_b = nc.gpsimd if (c & 1) == 0 else nc.vector
                        eng_a.tensor_scalar(
                            out=loh[:], in0=i128_sb[:],
                            scalar1=sloc_sb[:, c:c + 1], scalar2=None,
                            op0=mybir.AluOpType.is_equal,
                        )
                        eng_b.tensor_scalar(
                            out=roh[:, 0:hi - lo], in0=i512_sb[:, lo:hi],
                            scalar1=dloc_sb[:, c:c + 1], scalar2=None,
                            op0=mybir.AluOpType.is_equal,
                        )
                        nc.tensor.matmul(
                            pa[:, lo:hi], loh[:], roh[:, 0:hi - lo],
                            start=True, stop=True,
                        )
                        nc.tensor.matmul(
                            acc_ps[:, 32 + g:33 + g], loh[:], ones16[:],
                            start=(c == gs_first[g]), stop=(c == gs_last[g]),
                        )
                    nc.scalar.copy(
                        a_sb[g // 2][:, (g % 2) * NDP + w * 512:
                                     (g % 2) * NDP + (w + 1) * 512],
                        pa[:])
                    bi += 1
                # rowdeg for this gs group just closed: f[g] and xwf[g]
                # now, hidden under the remaining groups' build (zero-deg
                # genes have all-zero A rows, so f needs no mask)
                nc.vector.tensor_scalar(
                    out=f_sb[:, g:g + 1], in0=acc_ps[:, 32 + g:33 + g],
                    scalar1=1.0, scalar2=None,
                    op0=mybir.AluOpType.max,
                )
                nc.scalar.sqrt(f_sb[:, g:g + 1], f_sb[:, g:g + 1])
                nc.vector.reciprocal(f_sb[:, g:g + 1], f_sb[:, g:g + 1])
                nc.vector.tensor_scalar(
                    out=xw_sb[g][:], in0=xw_sb[g][:],
                    scalar1=f_sb[:, g:g + 1], scalar2=None,
                    op0=mybir.AluOpType.mult,
                )
                xf8 = xwf_sb[g // 2][:, (g % 2) * OC:(g % 2 + 1) * OC]
                nc.vector.tensor_copy(xf8, xw_sb[g][:])
                nc.vector.tensor_tensor(
                    out=xwr_sb[g // 2][:, (g % 2) * OC:(g % 2 + 1) * OC],
                    in0=xw_sb[g][:], in1=xf8,
                    op=mybir.AluOpType.subtract,
                )

            # coldeg matmuls first (tiny; A ready), transpose, stage early
            for w in range(NDW):
                for q in range(4):
                    col = w * 4 + q
                    off = w * 512 + q * 128
                    for gs in range(NGS):
                        nc.tensor.matmul(
                            acc_ps[:, col:col + 1],
                            a_sb[gs // 2][:, (gs % 2) * NDP + off:
                                          (gs % 2) * NDP + off + 128],
                            ones_f8[:],
                            start=(gs == 0),
                            stop=(gs == 3),
                        )
            cd_sb = cpool.tile([128, 32], F32)
            nc.vector.tensor_copy(cd_sb[:], acc_ps[:, 0:32])
            cdT_ps = ppool.tile([32, 128], F32, tag="cdT", bufs=1)
            nc.tensor.transpose(cdT_ps[:], cd_sb[:], ident_sb[:])
            cdT16 = cpool.tile([32, 128], F16)
            nc.vector.tensor_copy(cdT16[:], cdT_ps[:])
            nc.gpsimd.dma_start(
                staged[512::513, :].rearrange("a (b f) -> a b f", b=4),
                cdT16[:],
            )

            # ---- phase F: P = A^T @ xwf ----
            for w in range(NDW):
                p16 = wpool.tile([128, 4 * OC], F16, tag="p16", bufs=3,
                                 name=f"p16_{w}")
                for q in range(4):
                    pp = ppool.tile([128, OC], F32, tag="pp", bufs=2,
                                    name=f"pp{w}_{q}")
                    off = w * 512 + q * 128
                    for pr in range(2):
                        lhsT = a_sb[pr][:].rearrange(
                            "p (two d) -> p two d", two=2
                        )[:, :, off:off + 128]
                        for si, srcs in enumerate((xwf_sb, xwr_sb)):
                            rhs = srcs[pr][:].rearrange(
                                "p (two n) -> p two n", two=2
                            )
                            nc.tensor.matmul(
                                pp[:], lhsT, rhs,
                                start=(pr == 0 and si == 0),
                                stop=(pr == 1 and si == 1),
                                perf_mode=mybir.MatmulPerfMode.DoubleRow,
                            )
                    nc.scalar.copy(p16[:, q * OC:(q + 1) * OC], pp[:])
                    nc.sync.dma_start(
                        staged[513 * w + 128 * q:513 * w + 128 * (q + 1), :],
                        p16[:, q * OC:(q + 1) * OC],
                    )

            # ---- ReduceScatter (sums partials, core c gets its stripe) ----
            nc.gpsimd.collective_compute(
                "ReduceScatter",
                mybir.AluOpType.add,
                replica_groups=[list(range(CORES))],
                ins=[staged[:].opt()],
                outs=[rsout[:].opt()],
            )

            # ---- post: g scale + bias (zero-coldeg drugs have P=0) ----
            cdg16 = cpool.tile([128, 4], F16)
            nc.gpsimd.dma_start(
                cdg16[:],
                rsout[512:513, :].rearrange("r (q p) -> (r p) q", q=4),
            )
            g_sb = cpool.tile([128, 4], F32)
            nc.vector.tensor_scalar(
                out=g_sb[:], in0=cdg16[:], scalar1=1.0, scalar2=None,
                op0=mybir.AluOpType.max,
            )
            nc.scalar.sqrt(g_sb[:], g_sb[:])
            nc.vector.reciprocal(g_sb[:], g_sb[:])
            for q in range(4):
                pq = wpool.tile([128, OC], F16, tag="pq", bufs=4,
                                name=f"pq{q}")
                eng = nc.sync if q % 2 == 0 else nc.gpsimd
                eng.dma_start(pq[:], rsout[q * 128:(q + 1) * 128, :])
                og = wpool.tile([128, OC], F32, tag="og", bufs=4,
                                name=f"og{q}")
                nc.vector.scalar_tensor_tensor(
                    out=og[:], in0=pq[:], scalar=g_sb[:, q:q + 1],
                    in1=bias_sb[:],
                    op0=mybir.AluOpType.mult, op1=mybir.AluOpType.add,
                )
                oeng = nc.gpsimd if q % 2 == 0 else nc.sync
                oeng.dma_start(out[q * 128:(q + 1) * 128, :], og[:])

    nc.finalize()
    return nc


def make_in_maps(x, weight, bias, edge_index, chunks):
    """Host-side sharding/layout only: no arithmetic on tensor values."""
    x = np.asarray(x, dtype=np.float32)
    weight = np.ascontiguousarray(np.asarray(weight, dtype=np.float32))
    bias = np.asarray(bias, dtype=np.float32)
    ei = np.asarray(edge_index)
    s_all = ei[0].astype(np.int64)
    d_all = ei[1].astype(np.int64)

    nch = len(chunks)
    brep = np.ascontiguousarray(
        np.tile(bias[None, :], (128, 1)).astype(np.float32)
    )
    i128 = np.ascontiguousarray(
        np.tile(np.arange(128, dtype=np.float16)[None, :], (128, 1))
    )
    i512 = np.ascontiguousarray(
        np.tile(np.arange(512, dtype=np.float16)[None, :], (128, 1))
    )
    ident = np.eye(128, dtype=np.float32)

    in_maps = []
    for c in range(CORES):
        m = (s_all >= GW * c) & (s_all < GW * (c + 1))
        s = s_all[m] - GW * c          # [0, 512)
        d = d_all[m]
        gs = s >> 7
        dw = d >> 9
        dl = d & 511
        sl_arr = np.full((128, nch), -1.0, dtype=np.float32)
        dl_arr = np.full((128, nch), -1.0, dtype=np.float32)
        # order edges to match chunk structure
        for t, (g, w, lo, hi) in enumerate(chunks):
            sel = (gs == g) & (dw == w) & (dl >= lo) & (dl < hi)
            n = int(sel.sum())
            assert n <= 128, f"chunk overflow: {n}"
            sl_arr[:n, t] = (s[sel] - 128 * g).astype(np.float32)
            dl_arr[:n, t] = dl[sel].astype(np.float32)

        import ml_dtypes

        xsT = np.ascontiguousarray(
            x[GW * c:GW * (c + 1), :].T.astype(ml_dtypes.bfloat16)
        )

        in_maps.append(
            {
                "xT": xsT,
                "w": np.ascontiguousarray(weight.astype(ml_dtypes.bfloat16)),
                "brep": brep,
                "i128": i128,
                "i512": i512,
                "ident": ident,
                "sloc": np.ascontiguousarray(sl_arr),
                "dloc": np.ascontiguousarray(dl_arr),
            }
        )
    return in_maps


_NC = None
_CHUNKS = None
_KEY = None


def _get_nc(edge_index):
    global _NC, _CHUNKS, _KEY
    key = hash(np.asarray(edge_index).tobytes())
    if _NC is None or key != _KEY:
        _CHUNKS = build_structure(edge_index)
        _NC = build_nc(_CHUNKS)
        _KEY = key
    return _NC, _CHUNKS


def kernel(x, weight, bias, edge_index, **run_kwargs):
    from concourse.bass_utils import run_bass_kernel_spmd

    nc, chunks = _get_nc(edge_index)
    in_maps = make_in_maps(x, weight, bias, edge_index, chunks)
    res = run_bass_kernel_spmd(nc, in_maps, core_ids=list(range(CORES)),
                               **run_kwargs)
    outs = res.results if hasattr(res, "results") else res
    full = np.empty((NDP, OC), dtype=np.float32)
    for c in range(CORES):
        full[GW * c:GW * (c + 1)] = outs[c]["out"]
    full = full[:ND]
    if run_kwargs:
        return full, res
    return full


# revision 33
# speedup vs baseline: 1.0235x; 1.0054x over previous
"""BipartiteGCN message-passing kernel for 8 TRN2 NeuronCores.

Math:  out = D_c^{-1/2} A^T D_r^{-1/2} (x @ W) + b
where A[s, d] = multiplicity of edge (gene s, drug d), s, d in [0, 4000).

Strategy (gene-window sharding, single f16 ReduceScatter):
  - Core c owns gene window [512c, 512c+512).  It holds ALL edges whose src
    falls in its window, so row_deg is local (no collective needed for f).
  - xw_c = x_c @ W computed locally with bf16 matmuls (1 cyc/row, inputs
    host-cast to bf16), then scaled by f = rsqrt(max(row_deg,1)) per gene.
  - A_c [512 genes x 4096 drugs] built in SBUF from one-hot outer products
    on the PE.  Edges are bucketed by (gene subwindow gs in 4, drug window
    dw in 8); within a bucket they are sorted by dst and cut into <=128-edge
    chunks at dst-value boundaries.  Cut points are computed from the
    max-over-cores running counts, so all 8 cores share one SPMD module;
    chunk dst-spans tile [0,512) disjointly, so every A-build matmul is its
    own start&stop accumulation region (no psum pre-zeroing).  One-hot
    compares (DVE 4x mode, 0.26 ns/elem) are split between DVE and Pool.
  - P_c = A_c^T @ (f*xw_c) partials [4096 drugs x 512] via fp8e4 DoubleRow
    matmuls (2 gene layers per matmul at 0.5 cyc/row) with an fp8 residual
    correction pass (P += A^T @ fp8(xwf - fp8(xwf))) to recover precision;
    col_deg partial rows are packed into one staged [8*513, 512] f16 tensor;
    a single f16 ReduceScatter sums partials and hands core c its 513-row
    stripe (512 P rows + 1 col_deg row).
  - Post: g = rsqrt(max(col_deg,1)), out = g*P + bias; host concatenates.
    (Zero-degree genes/drugs have all-zero A rows/cols, so no rsqrt masks.)
"""

import sys

if "/opt/trn_rl_repo" not in sys.path:
    sys.path.insert(0, "/opt/trn_rl_repo")

import numpy as np

import concourse.bass as bass  # noqa: F401
import concourse.mybir as mybir
from concourse import bacc, tile

CORES = 8
ND = 4000               # number of drugs (dst ids; src gene ids share range)
NDP = 4096              # padded drug dim
GW = 512                # genes per core
NGS = 4                 # gene subwindows of 128
NDW = 8                 # drug windows of 512
IC = 1024
OC = 512
SROW = NDW * 513        # staged rows: per window 512 P rows + 1 coldeg row

F32 = mybir.dt.float32
F16 = mybir.dt.float16
BF16 = mybir.dt.bfloat16
F8 = mybir.dt.float8e4


def build_structure(edge_index):
    """Compile-time chunk structure shared by all cores (SPMD).

    Returns chunks: list of (gs, dw, lo, hi) in bucket order (gs outer,
    dw inner), where every core has <=128 edges with src in its (c, gs)
    subwindow and dst-local in [lo, hi).
    """
    s_all = np.asarray(edge_index[0], dtype=np.int64)
    d_all = np.asarray(edge_index[1], dtype=np.int64)
    assert s_all.min() >= 0 and s_all.max() < ND, "src ids out of range"
    assert d_all.min() >= 0 and d_all.max() < ND, "dst ids out of range"

    core = s_all >> 9                  # gene window of 512
    gs = (s_all >> 7) & 3              # subwindow of 128
    dw = d_all >> 9                    # drug window of 512
    dl = d_all & 511                   # dst-local
    # counts[core, gs, dw, dl]
    key = ((core * NGS + gs) * NDW + dw) * 512 + dl
    counts = np.bincount(key, minlength=CORES * NGS * NDW * 512).reshape(
        CORES, NGS, NDW, 512
    )

    chunks = []
    for g in range(NGS):
        for w in range(NDW):
            cnt = counts[:, g, w, :]            # [CORES, 512]
            assert cnt.max() <= 128, "single dst value multiplicity > 128"
            lo = 0
            acc = np.zeros(CORES, dtype=np.int64)
            for v in range(512):
                cv = cnt[:, v]
                if (acc + cv).max() > 128:
                    chunks.append((g, w, lo, v))
                    lo = v
                    acc = cv.copy()
                else:
                    acc += cv
            chunks.append((g, w, lo, 512))
    return chunks


def build_nc(chunks):
    nch = len(chunks)
    # first/last global chunk index per gs (for rowdeg psum start/stop)
    gs_first = {}
    gs_last = {}
    for i, (g, w, lo, hi) in enumerate(chunks):
        gs_first.setdefault(g, i)
        gs_last[g] = i

    nc = bacc.Bacc(
        None,
        target_bir_lowering=False,
        debug=False,
        num_devices=CORES,
    )

    xT = nc.dram_tensor("xT", [IC, GW], BF16, kind="ExternalInput")
    w_in = nc.dram_tensor("w", [IC, OC], BF16, kind="ExternalInput")
    brep = nc.dram_tensor("brep", [128, OC], F32, kind="ExternalInput")
    i128 = nc.dram_tensor("i128", [128, 128], F16, kind="ExternalInput")
    i512 = nc.dram_tensor("i512", [128, 512], F16, kind="ExternalInput")
    ident = nc.dram_tensor("ident", [128, 128], F32, kind="ExternalInput")
    sloc = nc.dram_tensor("sloc", [128, nch], F32, kind="ExternalInput")
    dloc = nc.dram_tensor("dloc", [128, nch], F32, kind="ExternalInput")
    out = nc.dram_tensor("out", [GW, OC], F32, kind="ExternalOutput")

    staged = nc.dram_tensor("staged", [SROW, OC], F16)
    rsout = nc.dram_tensor("rsout", [513, OC], F16)

    with tile.TileContext(nc) as tc:
        with (
            tc.tile_pool(name="const", bufs=1) as cpool,
            tc.tile_pool(name="work", bufs=2) as wpool,
            tc.tile_pool(name="apool", bufs=1) as apool,
            tc.tile_pool(name="psum", bufs=2, space="PSUM") as ppool,
            tc.tile_pool(name="psheld", bufs=1, space="PSUM") as hpool,
        ):
            # ---- constants: one-hot inputs first (SP + ACT queues),
            # ACT sqrt-table preload deferred so it can't block dloc ----
            sloc_sb = cpool.tile([128, nch], F32)
            nc.sync.dma_start(sloc_sb[:], sloc[:])
            i128_sb = cpool.tile([128, 128], F16)
            nc.scalar.dma_start(i128_sb[:], i128[:])
            dloc_sb = cpool.tile([128, nch], F32)
            nc.sync.dma_start(dloc_sb[:], dloc[:])
            i512_sb = cpool.tile([128, 512], F16)
            nc.scalar.dma_start(i512_sb[:], i512[:])
            ident_sb = cpool.tile([128, 128], F32)
            nc.sync.dma_start(ident_sb[:], ident[:])
            bias_sb = cpool.tile([128, OC], F32)
            nc.scalar.dma_start(bias_sb[:], brep[:])
            # preload the ACT sqrt function table off the critical path
            junk = cpool.tile([128, 1], F32)
            nc.vector.memset(junk[:], 4.0)
            nc.scalar.sqrt(junk[:], junk[:])
            ones16 = cpool.tile([128, 1], F16)
            nc.vector.memset(ones16[:], 1.0)
            ones_f8 = cpool.tile([128, 1], F8)
            nc.vector.memset(ones_f8[:], 1.0)

            # ---- phase B: xw = x_c @ W (bf16) ----
            xt_t = []
            w_t = []
            for kt in range(8):
                xt = wpool.tile([128, GW], BF16, tag="xT", bufs=8,
                                name=f"xt{kt}")
                wt = wpool.tile([128, OC], BF16, tag="w", bufs=8,
                                name=f"w{kt}")
                nc.sync.dma_start(xt[:], xT[kt * 128:(kt + 1) * 128, :])
                nc.sync.dma_start(wt[:], w_in[kt * 128:(kt + 1) * 128, :])
                xt_t.append(xt)
                w_t.append(wt)
            xw_sb = []
            for gs in range(NGS):
                pg = ppool.tile([128, OC], F32, tag="pg", bufs=1,
                                name=f"pg{gs}")
                for kt in range(8):
                    nc.tensor.matmul(
                        pg[:],
                        xt_t[kt][:, gs * 128:(gs + 1) * 128],
                        w_t[kt][:],
                        start=(kt == 0),
                        stop=(kt == 7),
                    )
                t = cpool.tile([128, OC], F32, name=f"xw{gs}")
                nc.scalar.copy(t[:], pg[:])
                xw_sb.append(t)

            # ---- phase D: A-build + rowdeg ----
            a_sb = [apool.tile([128, 2 * NDP], F8, name=f"A{pr}") for pr in range(2)]
            acc_ps = hpool.tile([128, 36], F32, name="accps")
            f_sb = cpool.tile([128, NGS], F32)
            xwf_sb = [cpool.tile([128, 2 * OC], F8, name=f"xwf{pr}")
                      for pr in range(2)]
            xwr_sb = [cpool.tile([128, 2 * OC], F8, name=f"xwr{pr}")
                      for pr in range(2)]
            ci = 0
            bi = 0
            for g in range(NGS):
                for w in range(NDW):
                    pa = ppool.tile([128, 512], F32, tag="pa", bufs=3,
                                    name=f"pa{bi}")
                    bspans = [c for c in chunks if c[0] == g and c[1] == w]
                    for (gg, ww, lo, hi) in bspans:
                        c = ci
                        ci += 1
                        loh = wpool.tile([128, 128], F16, tag="loh", bufs=32,
                                         name=f"loh{c}")
                        roh = wpool.tile([128, 512], F16, tag="roh", bufs=32,
                                         name=f"roh{c}")
                        eng_a = nc.vector if (c & 1) == 0 else nc.gpsimd
                        eng_b = nc.gpsimd if (c & 1) == 0 else nc.vector
                        eng_a.tensor_scalar(
                            out=loh[:], in0=i128_sb[:],
                            scalar1=sloc_sb[:, c:c + 1], scalar2=None,
                            op0=mybir.AluOpType.is_equal,
                        )
                        eng_b.tensor_scalar(
                            out=roh[:, 0:hi - lo], in0=i512_sb[:, lo:hi],
                            scalar1=dloc_sb[:, c:c + 1], scalar2=None,
                            op0=mybir.AluOpType.is_equal,
                        )
                        nc.tensor.matmul(
                            pa[:, lo:hi], loh[:], roh[:, 0:hi - lo],
                            start=True, stop=True,
                        )
                        nc.tensor.matmul(
                            acc_ps[:, 32 + g:33 + g], loh[:], ones16[:],
                            start=(c == gs_first[g]), stop=(c == gs_last[g]),
                        )
                    nc.scalar.copy(
                        a_sb[g // 2][:, (g % 2) * NDP + w * 512:
                                     (g % 2) * NDP + (w + 1) * 512],
                        pa[:])
                    bi += 1
                # rowdeg for this gs group just closed: f[g] and xwf[g]
                # now, hidden under the remaining groups' build (zero-deg
                # genes have all-zero A rows, so f needs no mask)
                nc.vector.tensor_scalar(
                    out=f_sb[:, g:g + 1], in0=acc_ps[:, 32 + g:33 + g],
                    scalar1=1.0, scalar2=None,
                    op0=mybir.AluOpType.max,
                )
                nc.scalar.sqrt(f_sb[:, g:g + 1], f_sb[:, g:g + 1])
                nc.vector.reciprocal(f_sb[:, g:g + 1], f_sb[:, g:g + 1])
                nc.vector.tensor_scalar(
                    out=xw_sb[g][:], in0=xw_sb[g][:],
                    scalar1=f_sb[:, g:g + 1], scalar2=None,
                    op0=mybir.AluOpType.mult,
                )
                xf8 = xwf_sb[g // 2][:, (g % 2) * OC:(g % 2 + 1) * OC]
                nc.vector.tensor_copy(xf8, xw_sb[g][:])
                nc.vector.tensor_tensor(
                    out=xwr_sb[g // 2][:, (g % 2) * OC:(g % 2 + 1) * OC],
                    in0=xw_sb[g][:], in1=xf8,
                    op=mybir.AluOpType.subtract,
                )

            # coldeg matmuls first (tiny; A ready), transpose, stage early
            for w in range(NDW):
                for q in range(4):
                    col = w * 4 + q
                    off = w * 512 + q * 128
                    for gs in range(NGS):
                        nc.tensor.matmul(
                            acc_ps[:, col:col + 1],
                            a_sb[gs // 2][:, (gs % 2) * NDP + off:
                                          (gs % 2) * NDP + off + 128],
                            ones_f8[:],
                            start=(gs == 0),
                            stop=(gs == 3),
                        )
            cd_sb = cpool.tile([128, 32], F32)
            nc.vector.tensor_copy(cd_sb[:], acc_ps[:, 0:32])
            cdT_ps = ppool.tile([32, 128], F32, tag="cdT", bufs=1)
            nc.tensor.transpose(cdT_ps[:], cd_sb[:], ident_sb[:])
            cdT16 = cpool.tile([32, 128], F16)
            nc.vector.tensor_copy(cdT16[:], cdT_ps[:])
            nc.gpsimd.dma_start(
                staged[512::513, :].rearrange("a (b f) -> a b f", b=4),
                cdT16[:],
            )

            # ---- phase F: P = A^T @ xwf ----
            for w in range(NDW):
                p16 = wpool.tile([128, 4 * OC], F16, tag="p16", bufs=3,
                                 name=f"p16_{w}")
                for q in range(4):
                    pp = ppool.tile([128, OC], F32, tag="pp", bufs=2,
                                    name=f"pp{w}_{q}")
                    off = w * 512 + q * 128
                    for pr in range(2):
                        lhsT = a_sb[pr][:].rearrange(
                            "p (two d) -> p two d", two=2
                        )[:, :, off:off + 128]
                        for si, srcs in enumerate((xwf_sb, xwr_sb)):
                            rhs = srcs[pr][:].rearrange(
                                "p (two n) -> p two n", two=2
                            )
                            nc.tensor.matmul(
                                pp[:], lhsT, rhs,
                                start=(pr == 0 and si == 0),
                                stop=(pr == 1 and si == 1),
                                perf_mode=mybir.MatmulPerfMode.DoubleRow,
                            )
                    nc.scalar.copy(p16[:, q * OC:(q + 1) * OC], pp[:])
                    nc.sync.dma_start(
                        staged[513 * w + 128 * q:513 * w + 128 * (q + 1), :],
                        p16[:, q * OC:(q + 1) * OC],
                    )

            # ---- ReduceScatter (sums partials, core c gets its stripe) ----
            nc.gpsimd.collective_compute(
                "ReduceScatter",
                mybir.AluOpType.add,
                replica_groups=[list(range(CORES))],
                ins=[staged[:].opt()],
                outs=[rsout[:].opt()],
            )

            # ---- post: g scale + bias (zero-coldeg drugs have P=0) ----
            cdg16 = cpool.tile([128, 4], F16)
            nc.gpsimd.dma_start(
                cdg16[:],
                rsout[512:513, :].rearrange("r (q p) -> (r p) q", q=4),
            )
            g_sb = cpool.tile([128, 4], F32)
            nc.vector.tensor_scalar(
                out=g_sb[:], in0=cdg16[:], scalar1=1.0, scalar2=None,
                op0=mybir.AluOpType.max,
            )
            nc.scalar.sqrt(g_sb[:], g_sb[:])
            nc.vector.reciprocal(g_sb[:], g_sb[:])
            for q in range(4):
                pq = wpool.tile([128, OC], F16, tag="pq", bufs=4,
                                name=f"pq{q}")
                eng = nc.sync if q % 2 == 0 else nc.gpsimd
                eng.dma_start(pq[:], rsout[q * 128:(q + 1) * 128, :])
                og = wpool.tile([128, OC], F32, tag="og", bufs=4,
                                name=f"og{q}")
                nc.vector.scalar_tensor_tensor(
                    out=og[:], in0=pq[:], scalar=g_sb[:, q:q + 1],
                    in1=bias_sb[:],
                    op0=mybir.AluOpType.mult, op1=mybir.AluOpType.add,
                )
                oeng = nc.gpsimd if q % 2 == 0 else nc.sync
                oeng.dma_start(out[q * 128:(q + 1) * 128, :], og[:])

    nc.finalize()
    return nc


def make_in_maps(x, weight, bias, edge_index, chunks):
    """Host-side sharding/layout only: no arithmetic on tensor values."""
    x = np.asarray(x, dtype=np.float32)
    weight = np.ascontiguousarray(np.asarray(weight, dtype=np.float32))
    bias = np.asarray(bias, dtype=np.float32)
    ei = np.asarray(edge_index)
    s_all = ei[0].astype(np.int64)
    d_all = ei[1].astype(np.int64)

    nch = len(chunks)
    brep = np.ascontiguousarray(
        np.tile(bias[None, :], (128, 1)).astype(np.float32)
    )
    i128 = np.ascontiguousarray(
        np.tile(np.arange(128, dtype=np.float16)[None, :], (128, 1))
    )
    i512 = np.ascontiguousarray(
        np.tile(np.arange(512, dtype=np.float16)[None, :], (128, 1))
    )
    ident = np.eye(128, dtype=np.float32)

    in_maps = []
    for c in range(CORES):
        m = (s_all >= GW * c) & (s_all < GW * (c + 1))
        s = s_all[m] - GW * c          # [0, 512)
        d = d_all[m]
        gs = s >> 7
        dw = d >> 9
        dl = d & 511
        sl_arr = np.full((128, nch), -1.0, dtype=np.float32)
        dl_arr = np.full((128, nch), -1.0, dtype=np.float32)
        # order edges to match chunk structure
        for t, (g, w, lo, hi) in enumerate(chunks):
            sel = (gs == g) & (dw == w) & (dl >= lo) & (dl < hi)
            n = int(sel.sum())
            assert n <= 128, f"chunk overflow: {n}"
            sl_arr[:n, t] = (s[sel] - 128 * g).astype(np.float32)
            dl_arr[:n, t] = dl[sel].astype(np.float32)

        import ml_dtypes

        xsT = np.ascontiguousarray(
            x[GW * c:GW * (c + 1), :].T.astype(ml_dtypes.bfloat16)
        )

        in_maps.append(
            {
                "xT": xsT,
                "w": np.ascontiguousarray(weight.astype(ml_dtypes.bfloat16)),
                "brep": brep,
                "i128": i128,
                "i512": i512,
                "ident": ident,
                "sloc": np.ascontiguousarray(sl_arr),
                "dloc": np.ascontiguousarray(dl_arr),
            }
        )
    return in_maps


_NC = None
_CHUNKS = None
_KEY = None


def _get_nc(edge_index):
    global _NC, _CHUNKS, _KEY
    key = hash(np.asarray(edge_index).tobytes())
    if _NC is None or key != _KEY:
        _CHUNKS = build_structure(edge_index)
        _NC = build_nc(_CHUNKS)
        _KEY = key
    return _NC, _CHUNKS


def kernel(x, weight, bias, edge_index, **run_kwargs):
    from concourse.bass_utils import run_bass_kernel_spmd

    nc, chunks = _get_nc(edge_index)
    in_maps = make_in_maps(x, weight, bias, edge_index, chunks)
    res = run_bass_kernel_spmd(nc, in_maps, core_ids=list(range(CORES)),
                               **run_kwargs)
    outs = res.results if hasattr(res, "results") else res
    full = np.empty((NDP, OC), dtype=np.float32)
    for c in range(CORES):
        full[GW * c:GW * (c + 1)] = outs[c]["out"]
    full = full[:ND]
    if run_kwargs:
        return full, res
    return full
